# revision 48
# baseline (speedup 1.0000x reference)
"""Trainium2 Bass kernel for nn_MultiHeadedAttention_25984552141341 (v2).

Computation (reference):
    q/k/v = (x @ W + b) split into 8 heads of 64
    scores = q @ k^T / 8
    scores += sf_net(scores)   (SoftmaxResNet over the key dim, 71)
    p = softmax(scores, axis=key)
    out = (p @ v merged) @ Wo + bo

Sharding: batch (512) split across 8 NeuronCores, 64 batches each.
All weights replicated; each core runs an identical Bass program (SPMD).

v2 changes vs the v1 baseline (573us):
  * host side: inputs/weights pre-cast to bf16 (halves HBM read traffic),
    1/sqrt(dh)=0.125 folded into Wq/bq, bo_eff = bo + bv @ Wo precomputed.
  * q/k projections computed over the full 4544-token range in 512-token
    slabs (large moving operands, fewer matmul instructions).
  * gelu/exp phase split: per group of 8 batches the scalar engine runs
    8 gelus back-to-back then 8 exps (2 ACT table loads per group instead
    of per pair) -- requires evicting scores to SBUF and re-adding the
    sf-net output (DVE) instead of accumulating w2 into the scores PSUM.
  * output projection emitted per group in the transposed orientation
    (outT = Wo^T @ attnT), so bo_eff rides the scalar PSUM->SBUF copy as
    a per-partition bias; result written to a [D, T] dram tensor and
    untransposed on the host.
  * engine queues staggered (PE runs phase A of group g+1 before phase B
    of group g) so the PE never waits on the scalar/DVE pipeline.
"""

import contextlib
import os

import numpy as np
import ml_dtypes

SCALT = os.environ.get("SCALT", "0") == "1"

import concourse.bass as bass
import concourse.mybir as mybir
import concourse.tile as tile
from concourse import bacc
from concourse import bass_utils
from concourse.masks import make_identity

F32 = mybir.dt.float32
BF16 = mybir.dt.bfloat16
AF = mybir.ActivationFunctionType

N_CORES = 8
B, L, D, H = 512, 71, 512, 8
DH = D // H  # 64
FF = 128  # sf_net hidden
BC = B // N_CORES  # 64 batches per core
T = BC * L  # 4544 tokens per core
GB = 8  # batches per group
G = BC // GB  # 8 groups
GT = GB * L  # 568 tokens per group
SLAB = 512  # token slab for q/k projections
NSLAB = (T + SLAB - 1) // SLAB  # 9 (last = 448)

_CACHE = {}


def _build():
    nc = bacc.Bacc("TRN2", target_bir_lowering=False, debug=False,
                   num_devices=N_CORES)

    xqT = nc.dram_tensor("xqT", [D, T], BF16, kind="ExternalInput").ap()
    xkT = nc.dram_tensor("xkT", [D, T], BF16, kind="ExternalInput").ap()
    xvT = nc.dram_tensor("xvT", [D, T], BF16, kind="ExternalInput").ap()
    Wq = nc.dram_tensor("Wq", [D, D], BF16, kind="ExternalInput").ap()
    Wk = nc.dram_tensor("Wk", [D, D], BF16, kind="ExternalInput").ap()
    Wv = nc.dram_tensor("Wv", [D, D], BF16, kind="ExternalInput").ap()
    Wo = nc.dram_tensor("Wo", [D, D], BF16, kind="ExternalInput").ap()
    bq = nc.dram_tensor("bq", [D], F32, kind="ExternalInput").ap()   # pre-scaled by 1/8
    bk = nc.dram_tensor("bk", [D], F32, kind="ExternalInput").ap()
    bo = nc.dram_tensor("bo", [D], F32, kind="ExternalInput").ap()   # bo_eff
    w1 = nc.dram_tensor("sf_w1", [L, FF], BF16, kind="ExternalInput").ap()
    b1 = nc.dram_tensor("sf_b1", [FF], F32, kind="ExternalInput").ap()
    w2 = nc.dram_tensor("sf_w2", [FF, L], BF16, kind="ExternalInput").ap()
    b2 = nc.dram_tensor("sf_b2", [L], F32, kind="ExternalInput").ap()
    outT_d = nc.dram_tensor("outT", [D, T], F32, kind="ExternalOutput").ap()

    with tile.TileContext(nc) as tc, contextlib.ExitStack() as ctx:
        singles = ctx.enter_context(tc.tile_pool(name="singles", bufs=1))
        p_xqk = ctx.enter_context(tc.tile_pool(name="xqk", bufs=3))
        p_xv = ctx.enter_context(tc.tile_pool(name="xv", bufs=2))
        # v/Ssb/gel live from phase A of group g across phase A of group g+1
        # (emitted before phase B of g) -> need 2*GB live buffers.
        p_v = ctx.enter_context(tc.tile_pool(name="v", bufs=2 * GB))
        p_ssb = ctx.enter_context(tc.tile_pool(name="ssb", bufs=2 * GB))
        p_s2 = ctx.enter_context(tc.tile_pool(name="s2", bufs=3))
        p_esb = ctx.enter_context(tc.tile_pool(name="esb", bufs=4))
        p_gel = ctx.enter_context(tc.tile_pool(name="gel", bufs=2 * GB))
        p_asc = ctx.enter_context(tc.tile_pool(name="asc", bufs=4))
        p_att = ctx.enter_context(tc.tile_pool(name="att", bufs=2))
        p_osb = ctx.enter_context(tc.tile_pool(name="osb", bufs=3))
        p_small = ctx.enter_context(tc.tile_pool(name="small", bufs=4))
        # PSUM: 8 banks total, two pools of 2 x 2-bank slots.
        ps_a = ctx.enter_context(tc.tile_pool(name="psa", bufs=2, space="PSUM"))
        ps_b = ctx.enter_context(tc.tile_pool(name="psb", bufs=2, space="PSUM"))

        # ---- constants / weights ----
        def w_tiles(w_ap, name, eng=None):
            t = singles.tile([128, 4, D], BF16, tag=f"w_{name}")
            # sync queue: runs in parallel with the input-slab DMAs on gpsimd
            (eng or nc.sync).dma_start(
                out=t, in_=w_ap.rearrange("(j p) d -> p j d", p=128))
            return t

        Wq_sb = w_tiles(Wq, "q")
        Wk_sb = w_tiles(Wk, "k")
        Wv_sb = w_tiles(Wv, "v")

        def b_tile(b_ap, name):
            t = singles.tile([128, 4], F32, tag=f"b_{name}")
            nc.sync.dma_start(out=t, in_=b_ap.rearrange("(j p) -> p j", p=128))
            return t

        bq_sb = b_tile(bq, "q")   # already includes the 0.125 score scale
        bk_sb = b_tile(bk, "k")
        boT_sb = singles.tile([128, 4], F32, tag="boT")  # bo_eff[128*dc + p]
        nc.sync.dma_start(out=boT_sb, in_=bo.rearrange("(j p) -> p j", p=128))

        qT = singles.tile([128, 4, T], BF16, tag="qT")
        kT = singles.tile([128, 4, T], BF16, tag="kT")

        xq3 = xqT.rearrange("(j p) t -> p j t", p=128)
        xk3 = xkT.rearrange("(j p) t -> p j t", p=128)
        xv3 = xvT.rearrange("(j p) t -> p j t", p=128)

        # ---- q/k projections over full T in 512-token slabs ----
        # qT/kT layout [p, dt, t]: output feature = 128*dt + p
        def emit_slab(s):
            t0 = s * SLAB
            w = min(SLAB, T - t0)
            xtq = p_xqk.tile([128, 4, SLAB], BF16, tag="xtq")
            xtk = p_xqk.tile([128, 4, SLAB], BF16, tag="xtk")
            # split the q/k input feeds across two DMA queues (SWDGE / HWDGE)
            nc.gpsimd.dma_start(out=xtq[:, :, 0:w], in_=xq3[:, :, t0:t0 + w])
            nc.sync.dma_start(out=xtk[:, :, 0:w], in_=xk3[:, :, t0:t0 + w])
            for dt_ in range(4):
                pq = ps_a.tile([128, SLAB], F32, tag="psa")
                for j in range(4):
                    nc.tensor.matmul(
                        pq[:, 0:w], Wq_sb[:, j, dt_ * 128:(dt_ + 1) * 128],
                        xtq[:, j, 0:w], start=(j == 0), stop=(j == 3))
                nc.scalar.activation(
                    out=qT[:, dt_, t0:t0 + w], in_=pq[:, 0:w], func=AF.Identity,
                    bias=bq_sb[:, dt_:dt_ + 1], scale=1.0)
                pk = ps_a.tile([128, SLAB], F32, tag="psa")
                for j in range(4):
                    nc.tensor.matmul(
                        pk[:, 0:w], Wk_sb[:, j, dt_ * 128:(dt_ + 1) * 128],
                        xtk[:, j, 0:w], start=(j == 0), stop=(j == 3))
                nc.vector.tensor_scalar_add(
                    out=kT[:, dt_, t0:t0 + w], in0=pk[:, 0:w],
                    scalar1=bk_sb[:, dt_:dt_ + 1])

        # ---- per-group state ----
        # Phase A of group g: v-proj, scores, cast, h1, gelu  (gelu LUT set)
        # Phase B of group g: w2, S2-add, exp, attn, normalize, transpose
        # Emission per iteration: A(g+1) first, then B(g), then out-proj(g):
        # the PE chews A(g+1) while scalar/DVE finish B(g)'s chains.
        st = {}  # per-batch state: (Ssb, gel, v_sb)
        att_seg = {}  # per-group attnT segment
        xtv_tiles = {}

        def ensure_xtv(g):
            if g >= G or g in xtv_tiles:
                return
            t0g = g * GT
            xtv = p_xv.tile([128, 4, GT], BF16, tag="xtv")
            nc.gpsimd.dma_start(out=xtv, in_=xv3[:, :, t0g:t0g + GT])
            xtv_tiles[g] = xtv

        def emit_phase_a(g):
            t0g = g * GT
            ensure_xtv(g)
            ensure_xtv(g + 1)  # prefetch one group ahead of the PE
            xtv = xtv_tiles.pop(g)
            prev = None
            for bl in range(GB):
                tc0 = t0g + bl * L
                bi = g * GB + bl
                # scores S' = [k, q]; head h = 2*hh + i lives at col 512*i + 71*hh
                # (emitted before the v-projection: the group's first PE op
                # then has zero pending dependencies at the B->A seam, while
                # the v-proj's PSUM slot wait on the previous group's DVE
                # tail is absorbed by the scores matmuls)
                S_ps = ps_a.tile([L, 1024], F32, tag="psa")
                if SCALT:
                    # independent single-matmul groups, even/odd interleaved:
                    # adjacent matmuls use disjoint PE row-groups (contraction
                    # rows 0-63 vs 64-127) and different PSUM banks, so the
                    # array runs two heads concurrently.
                    for hh in range(4):
                        for i in range(2):
                            off = 512 * i + L * hh
                            nc.tensor.matmul(
                                S_ps[:, off:off + L],
                                kT[64 * i:64 * i + 64, hh, tc0:tc0 + L],
                                qT[64 * i:64 * i + 64, hh, tc0:tc0 + L],
                                start=True, stop=True)
                else:
                    for i in range(2):
                        for hh in range(4):
                            off = 512 * i + L * hh
                            nc.tensor.matmul(
                                S_ps[:, off:off + L],
                                kT[64 * i:64 * i + 64, hh, tc0:tc0 + L],
                                qT[64 * i:64 * i + 64, hh, tc0:tc0 + L],
                                start=(hh == 0), stop=(hh == 3))
                Ssb = p_ssb.tile([L, 2, 4 * L], BF16, tag="Ssb")
                nc.scalar.activation(
                    out=Ssb, in_=S_ps.rearrange("p (b r) -> p b r", b=2)[:, :, 0:4 * L],
                    func=AF.Identity, scale=1.0)
                # v projection: stationary x-slice, moving Wv -> [tok, D]
                pv = ps_b.tile([L, 1024], F32, tag="psb")
                for j in range(4):
                    nc.tensor.matmul(pv[:, 0:D], xtv[:, j, bl * L:bl * L + L],
                                     Wv_sb[:, j, :], start=(j == 0), stop=(j == 3))
                v_sb = p_v.tile([L, H, DH + 1], BF16, tag="v")
                nc.gpsimd.memset(v_sb[:, :, DH:DH + 1], 1.0)
                nc.vector.tensor_copy(
                    out=v_sb[:, :, 0:DH],
                    in_=pv[:, 0:D].rearrange("p (h d) -> p h d", h=H))
                st[bi] = [Ssb, None, v_sb]
                # h1 + gelu for the PREVIOUS batch (stagger: the cast of batch
                # bl completes while the PE runs v/scores of batch bl+1)
                if prev is not None:
                    emit_h1_gelu(prev)
                prev = bi
            emit_h1_gelu(prev)

        def emit_h1_gelu(bi):
            Ssb = st[bi][0]
            Sflat = Ssb.rearrange("p b r -> p (b r)")
            h1_ps = ps_a.tile([FF, 1024], F32, tag="psa")
            nc.tensor.matmul(h1_ps[:, 0:512], w1_sb, Sflat[:, 0:512],
                             start=True, stop=True)
            nc.tensor.matmul(h1_ps[:, 512:GT], w1_sb, Sflat[:, 512:GT],
                             start=True, stop=True)
            gel = p_gel.tile([FF, GT], BF16, tag="gel")
            nc.scalar.activation(out=gel, in_=h1_ps[:, 0:GT], func=AF.Gelu,
                                 bias=b1_sb, scale=1.0)
            st[bi][1] = gel

        def emit_attn(g, bl, seg):
            bi = g * GB + bl
            E_sb, v_sb = st.pop(bi)
            # attention + denominators; E col block p hosts head
            # h = 2*(p%4) + p//4 (from the scores layout)
            pa = ps_b.tile([L, 1024], F32, tag="psb")
            for p in range(H):
                h = 2 * (p % 4) + (p // 4)
                off = 512 * (p // 4) + (DH + 1) * (p % 4)
                nc.tensor.matmul(
                    pa[:, off:off + DH + 1],
                    E_sb[:, L * p:L * p + L], v_sb[:, h, :],
                    start=(p % 4 == 0), stop=(p % 4 == 3))
            return (bl, pa)

        def emit_phase_b(g, out_group=None):
            # Deep stagger: attn runs 2 batches behind the w2->S2->exp chain,
            # normalize (DVE) 3 behind, transpose 4 behind -- every PE op's
            # inputs are ready >=2 batch-slots before the strict-FIFO PE
            # reaches it.  Per-bl emission order (h2, norm, transpose, attn,
            # out-chunk) also keeps every PSUM ring-slot reuse behind the
            # previous occupant's already-emitted accesses.
            seg = p_att.tile([128, 4, GT], BF16, tag="attseg")
            att_seg[g] = seg
            pa_q, asc_q = [], []
            for bl in range(GB):
                bi = g * GB + bl
                Ssb, gel, v_sb = st.pop(bi)
                Sflat = Ssb.rearrange("p b r -> p (b r)")
                # sf-net output layer into a fresh PSUM, then S2 = S + h2 (DVE)
                h2_ps = ps_b.tile([L, 1024], F32, tag="psb")
                nc.tensor.matmul(h2_ps[:, 0:512], w2_sb, gel[:, 0:512],
                                 start=True, stop=True)
                nc.tensor.matmul(h2_ps[:, 512:GT], w2_sb, gel[:, 512:GT],
                                 start=True, stop=True)
                S2 = p_s2.tile([L, GT], BF16, tag="S2")
                nc.vector.tensor_add(S2, h2_ps[:, 0:GT], Sflat)
                # softmax numerator (no max subtraction; |scores2| < ~4)
                E_sb = p_esb.tile([L, GT], BF16, tag="E")
                nc.scalar.activation(out=E_sb, in_=S2, func=AF.Exp,
                                     bias=b2_sb, scale=1.0)
                st[bi] = (E_sb, v_sb)
                if bl >= 3:
                    asc_q.append(emit_norm(g, *pa_q.pop(0)))
                if bl >= 4:
                    emit_transpose(g, *asc_q.pop(0), seg)
                if bl >= 2:
                    pa_q.append(emit_attn(g, bl - 2, seg))
                if out_group is not None and bl % 2 == 1 and bl >= 3:
                    emit_out_chunk(out_group, bl // 2 - 1)
            # tail: drain the stagger queues.  norm(b) must be emitted before
            # the attn that recycles pa(b)'s ring slot; real matmuls sit
            # between the transpose clusters so the HAM activity monitor
            # never sees an idle window, and the last out chunk carries the
            # PE across the B->A seam while the DVE tail frees the next
            # group's PSUM slots.
            # the last out chunk leads the tail: ~2us of zero-dependency PE
            # filler that absorbs the exp(6)/exp(7) queue lag before the tail
            # attns (the B->A seam no longer needs it -- phase A now opens
            # with zero-dependency scores matmuls)
            if out_group is not None:
                emit_out_chunk(out_group, 3)
            pa_q.append(emit_attn(g, GB - 2, seg))          # attn(6)
            asc_q.append(emit_norm(g, *pa_q.pop(0)))        # norm(5)
            emit_transpose(g, *asc_q.pop(0), seg)           # tp(4)
            asc_q.append(emit_norm(g, *pa_q.pop(0)))        # norm(6)
            pa_q.append(emit_attn(g, GB - 1, seg))          # attn(7)
            asc_q.append(emit_norm(g, *pa_q.pop(0)))        # norm(7)
            emit_transpose(g, *asc_q.pop(0), seg)           # tp(5)
            emit_transpose(g, *asc_q.pop(0), seg)           # tp(6)
            emit_transpose(g, *asc_q.pop(0), seg)           # tp(7)

        def emit_norm(g, bl, pa):
            recip = p_small.tile([L, 2, 4], F32, tag="recip")
            # denominators live at col 512*bnk + 65*h + 64
            nc.vector.reciprocal(
                out=recip,
                in_=bass.AP(tensor=pa.tensor, offset=pa.offset + DH,
                            ap=[pa.ap[0], [512, 2], [DH + 1, 4]]))
            # scale + cast; bank b's blocks (heads 2*hh+b) go to col 128*hh+64*b
            # (single 4D-AP op: [part, bank, head, elem])
            asc = p_asc.tile([L, D], BF16, tag="asc")
            nc.vector.tensor_mul(
                bass.AP(tensor=asc.tensor, offset=asc.offset,
                        ap=[asc.ap[0], [DH, 2], [2 * DH, 4], [1, DH]]),
                bass.AP(tensor=pa.tensor, offset=pa.offset,
                        ap=[pa.ap[0], [512, 2], [DH + 1, 4], [1, DH]]),
                bass.AP(tensor=recip.tensor, offset=recip.offset,
                        ap=[recip.ap[0], [4, 2], [1, 4], [0, DH]]))
            return (bl, asc)

        def emit_transpose(g, bl, asc, seg):
            # transpose attn rows to [feat, tok] into the group attnT segment
            tp = ps_b.tile([128, 4, L + 1], BF16, tag="psb")
            for j in range(4):
                nc.tensor.transpose(tp[:, j, 0:L],
                                    asc[:, 128 * j:128 * (j + 1)], ident)
            nc.vector.tensor_copy(
                out=seg[:, :, bl * L:(bl + 1) * L], in_=tp[:, :, 0:L])

        def emit_out_chunk(g, dc):
            seg = att_seg[g]
            t0g = g * GT
            po = ps_a.tile([128, 1024], F32, tag="psa")
            for j in range(4):
                nc.tensor.matmul(
                    po[:, 0:512], Wo_sb[:, j, dc * 128:(dc + 1) * 128],
                    seg[:, j, 0:512], start=(j == 0), stop=(j == 3))
                nc.tensor.matmul(
                    po[:, 512:GT], Wo_sb[:, j, dc * 128:(dc + 1) * 128],
                    seg[:, j, 512:GT], start=(j == 0), stop=(j == 3))
            osb = p_osb.tile([128, GT], F32, tag="osb")
            nc.scalar.activation(out=osb, in_=po[:, 0:GT], func=AF.Identity,
                                 bias=boT_sb[:, dc:dc + 1], scale=1.0)
            nc.sync.dma_start(
                out=outT_d[dc * 128:(dc + 1) * 128, t0g:t0g + GT], in_=osb)
            if dc == 3:
                att_seg.pop(g)

        # ---- emission schedule ----
        # Group 0's phase A only needs q/k for tokens 0..568 (slabs 0-1) and
        # xtv(0): emit it right after slab 1 so the PE has ~14us of ready
        # work while slabs 2-8 stream in (the startup was DMA-feed-paced).
        emit_slab(0)
        emit_slab(1)
        ensure_xtv(0)
        ensure_xtv(1)
        w1_sb = singles.tile([L, FF], BF16, tag="w1")
        nc.gpsimd.dma_start(out=w1_sb, in_=w1)
        b1_sb = singles.tile([FF, 1], F32, tag="b1")
        nc.gpsimd.dma_start(out=b1_sb, in_=b1.rearrange("(p o) -> p o", o=1))
        emit_phase_a(0)
        for s in range(2, NSLAB):
            emit_slab(s)
        # constants not needed until phase B / out-proj
        Wo_sb = w_tiles(Wo, "o")
        w2_sb = singles.tile([FF, L], BF16, tag="w2")
        nc.gpsimd.dma_start(out=w2_sb, in_=w2)
        b2_sb = singles.tile([L, 1], F32, tag="b2")
        nc.gpsimd.dma_start(out=b2_sb, in_=b2.rearrange("(p o) -> p o", o=1))
        ident = singles.tile([L, L], BF16, tag="ident")
        make_identity(nc, ident)

        for g in range(G):
            if g + 1 < G:
                emit_phase_a(g + 1)
            emit_phase_b(g, out_group=(g - 1 if g >= 1 else None))
        for dc in range(4):
            emit_out_chunk(G - 1, dc)

    nc.compile()
    return nc


def _get_nc():
    if "nc" not in _CACHE:
        _CACHE["nc"] = _build()
    return _CACHE["nc"]


def _prep_in_maps(inputs):
    BF = ml_dtypes.bfloat16
    f32 = lambda a: np.asarray(a, dtype=np.float32)
    Wq_s = f32(inputs["Wq"]) * 0.125
    bq_s = f32(inputs["bq"]) * 0.125
    bo_eff = f32(inputs["bo"]) + f32(inputs["bv"]) @ f32(inputs["Wo"])
    shared = {
        "Wq": np.ascontiguousarray(Wq_s.astype(BF)),
        "Wk": np.ascontiguousarray(f32(inputs["Wk"]).astype(BF)),
        "Wv": np.ascontiguousarray(f32(inputs["Wv"]).astype(BF)),
        "Wo": np.ascontiguousarray(f32(inputs["Wo"]).astype(BF)),
        "bq": np.ascontiguousarray(bq_s),
        "bk": np.ascontiguousarray(f32(inputs["bk"])),
        "bo": np.ascontiguousarray(bo_eff),
        "sf_w1": np.ascontiguousarray(f32(inputs["sf_w1"]).astype(BF)),
        "sf_b1": np.ascontiguousarray(f32(inputs["sf_b1"])),
        "sf_w2": np.ascontiguousarray(f32(inputs["sf_w2"]).astype(BF)),
        "sf_b2": np.ascontiguousarray(f32(inputs["sf_b2"])),
    }
    xT = {}
    for key, name in (("query", "xqT"), ("key", "xkT"), ("value", "xvT")):
        # [B, L, D] -> [D, B, L] feature-major, bf16
        xT[name] = f32(inputs[key]).transpose(2, 0, 1).astype(BF)
    in_maps = []
    for c in range(N_CORES):
        m = dict(shared)
        for name in ("xqT", "xkT", "xvT"):
            m[name] = np.ascontiguousarray(
                xT[name][:, c * BC:(c + 1) * BC, :]).reshape(D, T)
        in_maps.append(m)
    return in_maps


def run(inputs, trace=False):
    nc = _get_nc()
    in_maps = _prep_in_maps(inputs)
    res = bass_utils.run_bass_kernel_spmd(
        nc, in_maps, core_ids=list(range(N_CORES)), trace=trace)
    out = np.concatenate(
        [np.asarray(res.results[c]["outT"], dtype=np.float32)
         .reshape(D, BC, L).transpose(1, 2, 0) for c in range(N_CORES)],
        axis=0)
    return out, res


def kernel(**inputs) -> np.ndarray:
    out, _ = run(inputs, trace=False)
    return out


# revision 50
# speedup vs baseline: 1.0073x; 1.0073x over previous
"""Trainium2 Bass kernel for nn_MultiHeadedAttention_25984552141341 (v2).

Computation (reference):
    q/k/v = (x @ W + b) split into 8 heads of 64
    scores = q @ k^T / 8
    scores += sf_net(scores)   (SoftmaxResNet over the key dim, 71)
    p = softmax(scores, axis=key)
    out = (p @ v merged) @ Wo + bo

Sharding: batch (512) split across 8 NeuronCores, 64 batches each.
All weights replicated; each core runs an identical Bass program (SPMD).

v2 changes vs the v1 baseline (573us):
  * host side: inputs/weights pre-cast to bf16 (halves HBM read traffic),
    1/sqrt(dh)=0.125 folded into Wq/bq, bo_eff = bo + bv @ Wo precomputed.
  * q/k projections computed over the full 4544-token range in 512-token
    slabs (large moving operands, fewer matmul instructions).
  * gelu/exp phase split: per group of 8 batches the scalar engine runs
    8 gelus back-to-back then 8 exps (2 ACT table loads per group instead
    of per pair) -- requires evicting scores to SBUF and re-adding the
    sf-net output (DVE) instead of accumulating w2 into the scores PSUM.
  * output projection emitted per group in the transposed orientation
    (outT = Wo^T @ attnT), so bo_eff rides the scalar PSUM->SBUF copy as
    a per-partition bias; result written to a [D, T] dram tensor and
    untransposed on the host.
  * engine queues staggered (PE runs phase A of group g+1 before phase B
    of group g) so the PE never waits on the scalar/DVE pipeline.
"""

import contextlib
import os

import numpy as np
import ml_dtypes

SCALT = os.environ.get("SCALT", "0") == "1"

import concourse.bass as bass
import concourse.mybir as mybir
import concourse.tile as tile
from concourse import bacc
from concourse import bass_utils
from concourse.masks import make_identity

F32 = mybir.dt.float32
BF16 = mybir.dt.bfloat16
AF = mybir.ActivationFunctionType

N_CORES = 8
B, L, D, H = 512, 71, 512, 8
DH = D // H  # 64
FF = 128  # sf_net hidden
BC = B // N_CORES  # 64 batches per core
T = BC * L  # 4544 tokens per core
GB = 8  # batches per group
G = BC // GB  # 8 groups
GT = GB * L  # 568 tokens per group
SLAB = 512  # token slab for q/k projections
NSLAB = (T + SLAB - 1) // SLAB  # 9 (last = 448)

_CACHE = {}


def _build():
    nc = bacc.Bacc("TRN2", target_bir_lowering=False, debug=False,
                   num_devices=N_CORES)

    xqT = nc.dram_tensor("xqT", [D, T], BF16, kind="ExternalInput").ap()
    xkT = nc.dram_tensor("xkT", [D, T], BF16, kind="ExternalInput").ap()
    xvT = nc.dram_tensor("xvT", [D, T], BF16, kind="ExternalInput").ap()
    Wq = nc.dram_tensor("Wq", [D, D], BF16, kind="ExternalInput").ap()
    Wk = nc.dram_tensor("Wk", [D, D], BF16, kind="ExternalInput").ap()
    Wv = nc.dram_tensor("Wv", [D, D], BF16, kind="ExternalInput").ap()
    Wo = nc.dram_tensor("Wo", [D, D], BF16, kind="ExternalInput").ap()
    bq = nc.dram_tensor("bq", [D], F32, kind="ExternalInput").ap()   # pre-scaled by 1/8
    bk = nc.dram_tensor("bk", [D], F32, kind="ExternalInput").ap()
    bo = nc.dram_tensor("bo", [D], F32, kind="ExternalInput").ap()   # bo_eff
    w1 = nc.dram_tensor("sf_w1", [L, FF], BF16, kind="ExternalInput").ap()
    b1 = nc.dram_tensor("sf_b1", [FF], F32, kind="ExternalInput").ap()
    w2 = nc.dram_tensor("sf_w2", [FF, L], BF16, kind="ExternalInput").ap()
    b2 = nc.dram_tensor("sf_b2", [L], F32, kind="ExternalInput").ap()
    outT_d = nc.dram_tensor("outT", [D, T], F32, kind="ExternalOutput").ap()

    with tile.TileContext(nc) as tc, contextlib.ExitStack() as ctx:
        singles = ctx.enter_context(tc.tile_pool(name="singles", bufs=1))
        p_xqk = ctx.enter_context(tc.tile_pool(name="xqk", bufs=3))
        p_xv = ctx.enter_context(tc.tile_pool(name="xv", bufs=2))
        # v/Ssb/gel live from phase A of group g across phase A of group g+1
        # (emitted before phase B of g) -> need 2*GB live buffers.
        p_v = ctx.enter_context(tc.tile_pool(name="v", bufs=2 * GB))
        p_ssb = ctx.enter_context(tc.tile_pool(name="ssb", bufs=2 * GB))
        p_s2 = ctx.enter_context(tc.tile_pool(name="s2", bufs=3))
        p_esb = ctx.enter_context(tc.tile_pool(name="esb", bufs=4))
        p_gel = ctx.enter_context(tc.tile_pool(name="gel", bufs=2 * GB))
        p_asc = ctx.enter_context(tc.tile_pool(name="asc", bufs=4))
        p_att = ctx.enter_context(tc.tile_pool(name="att", bufs=2))
        p_osb = ctx.enter_context(tc.tile_pool(name="osb", bufs=3))
        p_small = ctx.enter_context(tc.tile_pool(name="small", bufs=4))
        # PSUM: 8 banks total, two pools of 2 x 2-bank slots.
        ps_a = ctx.enter_context(tc.tile_pool(name="psa", bufs=2, space="PSUM"))
        ps_b = ctx.enter_context(tc.tile_pool(name="psb", bufs=2, space="PSUM"))

        # ---- constants / weights ----
        def w_tiles(w_ap, name, eng=None):
            t = singles.tile([128, 4, D], BF16, tag=f"w_{name}")
            # sync queue: runs in parallel with the input-slab DMAs on gpsimd
            (eng or nc.sync).dma_start(
                out=t, in_=w_ap.rearrange("(j p) d -> p j d", p=128))
            return t

        Wq_sb = w_tiles(Wq, "q")
        Wk_sb = w_tiles(Wk, "k")
        Wv_sb = w_tiles(Wv, "v")

        def b_tile(b_ap, name):
            t = singles.tile([128, 4], F32, tag=f"b_{name}")
            nc.sync.dma_start(out=t, in_=b_ap.rearrange("(j p) -> p j", p=128))
            return t

        bq_sb = b_tile(bq, "q")   # already includes the 0.125 score scale
        bk_sb = b_tile(bk, "k")
        boT_sb = singles.tile([128, 4], F32, tag="boT")  # bo_eff[128*dc + p]
        nc.sync.dma_start(out=boT_sb, in_=bo.rearrange("(j p) -> p j", p=128))

        qT = singles.tile([128, 4, T], BF16, tag="qT")
        kT = singles.tile([128, 4, T], BF16, tag="kT")

        xq3 = xqT.rearrange("(j p) t -> p j t", p=128)
        xk3 = xkT.rearrange("(j p) t -> p j t", p=128)
        xv3 = xvT.rearrange("(j p) t -> p j t", p=128)

        # ---- q/k projections over full T in 512-token slabs ----
        # qT/kT layout [p, dt, t]: output feature = 128*dt + p
        def emit_slab(s):
            t0 = s * SLAB
            w = min(SLAB, T - t0)
            xtq = p_xqk.tile([128, 4, SLAB], BF16, tag="xtq")
            xtk = p_xqk.tile([128, 4, SLAB], BF16, tag="xtk")
            # split the q/k input feeds across two DMA queues (SWDGE / HWDGE)
            nc.gpsimd.dma_start(out=xtq[:, :, 0:w], in_=xq3[:, :, t0:t0 + w])
            nc.sync.dma_start(out=xtk[:, :, 0:w], in_=xk3[:, :, t0:t0 + w])
            for dt_ in range(4):
                pq = ps_a.tile([128, SLAB], F32, tag="psa")
                for j in range(4):
                    nc.tensor.matmul(
                        pq[:, 0:w], Wq_sb[:, j, dt_ * 128:(dt_ + 1) * 128],
                        xtq[:, j, 0:w], start=(j == 0), stop=(j == 3))
                nc.scalar.activation(
                    out=qT[:, dt_, t0:t0 + w], in_=pq[:, 0:w], func=AF.Identity,
                    bias=bq_sb[:, dt_:dt_ + 1], scale=1.0)
                pk = ps_a.tile([128, SLAB], F32, tag="psa")
                for j in range(4):
                    nc.tensor.matmul(
                        pk[:, 0:w], Wk_sb[:, j, dt_ * 128:(dt_ + 1) * 128],
                        xtk[:, j, 0:w], start=(j == 0), stop=(j == 3))
                nc.vector.tensor_scalar_add(
                    out=kT[:, dt_, t0:t0 + w], in0=pk[:, 0:w],
                    scalar1=bk_sb[:, dt_:dt_ + 1])

        # ---- per-group state ----
        # Phase A of group g: v-proj, scores, cast, h1, gelu  (gelu LUT set)
        # Phase B of group g: w2, S2-add, exp, attn, normalize, transpose
        # Emission per iteration: A(g+1) first, then B(g), then out-proj(g):
        # the PE chews A(g+1) while scalar/DVE finish B(g)'s chains.
        st = {}  # per-batch state: (Ssb, gel, v_sb)
        att_seg = {}  # per-group attnT segment
        xtv_tiles = {}

        def ensure_xtv(g):
            if g >= G or g in xtv_tiles:
                return
            t0g = g * GT
            xtv = p_xv.tile([128, 4, GT], BF16, tag="xtv")
            nc.gpsimd.dma_start(out=xtv, in_=xv3[:, :, t0g:t0g + GT])
            xtv_tiles[g] = xtv

        def emit_phase_a(g):
            t0g = g * GT
            ensure_xtv(g)
            ensure_xtv(g + 1)  # prefetch one group ahead of the PE
            xtv = xtv_tiles.pop(g)
            prev = None
            for bl in range(GB):
                tc0 = t0g + bl * L
                bi = g * GB + bl
                # scores S' = [k, q]; head h = 2*hh + i lives at col 512*i + 71*hh
                # (emitted before the v-projection: the group's first PE op
                # then has zero pending dependencies at the B->A seam, while
                # the v-proj's PSUM slot wait on the previous group's DVE
                # tail is absorbed by the scores matmuls)
                S_ps = ps_a.tile([L, 1024], F32, tag="psa")
                if SCALT:
                    # independent single-matmul groups, even/odd interleaved:
                    # adjacent matmuls use disjoint PE row-groups (contraction
                    # rows 0-63 vs 64-127) and different PSUM banks, so the
                    # array runs two heads concurrently.
                    for hh in range(4):
                        for i in range(2):
                            off = 512 * i + L * hh
                            nc.tensor.matmul(
                                S_ps[:, off:off + L],
                                kT[64 * i:64 * i + 64, hh, tc0:tc0 + L],
                                qT[64 * i:64 * i + 64, hh, tc0:tc0 + L],
                                start=True, stop=True)
                else:
                    for i in range(2):
                        for hh in range(4):
                            off = 512 * i + L * hh
                            nc.tensor.matmul(
                                S_ps[:, off:off + L],
                                kT[64 * i:64 * i + 64, hh, tc0:tc0 + L],
                                qT[64 * i:64 * i + 64, hh, tc0:tc0 + L],
                                start=(hh == 0), stop=(hh == 3))
                Ssb = p_ssb.tile([L, 2, 4 * L], BF16, tag="Ssb")
                nc.scalar.activation(
                    out=Ssb, in_=S_ps.rearrange("p (b r) -> p b r", b=2)[:, :, 0:4 * L],
                    func=AF.Identity, scale=1.0)
                # v projection: stationary x-slice, moving Wv -> [tok, D]
                pv = ps_b.tile([L, 1024], F32, tag="psb")
                for j in range(4):
                    nc.tensor.matmul(pv[:, 0:D], xtv[:, j, bl * L:bl * L + L],
                                     Wv_sb[:, j, :], start=(j == 0), stop=(j == 3))
                v_sb = p_v.tile([L, H, DH + 1], BF16, tag="v")
                nc.gpsimd.memset(v_sb[:, :, DH:DH + 1], 1.0)
                nc.vector.tensor_copy(
                    out=v_sb[:, :, 0:DH],
                    in_=pv[:, 0:D].rearrange("p (h d) -> p h d", h=H))
                st[bi] = [Ssb, None, v_sb]
                # h1 + gelu for the PREVIOUS batch (stagger: the cast of batch
                # bl completes while the PE runs v/scores of batch bl+1)
                if prev is not None:
                    emit_h1_gelu(prev)
                prev = bi
            emit_h1_gelu(prev)

        def emit_h1_gelu(bi):
            Ssb = st[bi][0]
            Sflat = Ssb.rearrange("p b r -> p (b r)")
            h1_ps = ps_a.tile([FF, 1024], F32, tag="psa")
            nc.tensor.matmul(h1_ps[:, 0:512], w1_sb, Sflat[:, 0:512],
                             start=True, stop=True)
            nc.tensor.matmul(h1_ps[:, 512:GT], w1_sb, Sflat[:, 512:GT],
                             start=True, stop=True)
            gel = p_gel.tile([FF, GT], BF16, tag="gel")
            nc.scalar.activation(out=gel, in_=h1_ps[:, 0:GT], func=AF.Gelu,
                                 bias=b1_sb, scale=1.0)
            st[bi][1] = gel

        def emit_attn(g, bl, seg):
            bi = g * GB + bl
            E_sb, v_sb = st.pop(bi)
            # attention + denominators; E col block p hosts head
            # h = 2*(p%4) + p//4 (from the scores layout)
            pa = ps_b.tile([L, 1024], F32, tag="psb")
            for p in range(H):
                h = 2 * (p % 4) + (p // 4)
                off = 512 * (p // 4) + (DH + 1) * (p % 4)
                nc.tensor.matmul(
                    pa[:, off:off + DH + 1],
                    E_sb[:, L * p:L * p + L], v_sb[:, h, :],
                    start=(p % 4 == 0), stop=(p % 4 == 3))
            return (bl, pa)

        def emit_phase_b(g, out_group=None):
            # Deep stagger: attn runs 2 batches behind the w2->S2->exp chain,
            # normalize (DVE) 3 behind, transpose 4 behind -- every PE op's
            # inputs are ready >=2 batch-slots before the strict-FIFO PE
            # reaches it.  Per-bl emission order (h2, norm, transpose, attn,
            # out-chunk) also keeps every PSUM ring-slot reuse behind the
            # previous occupant's already-emitted accesses.
            seg = p_att.tile([128, 4, GT], BF16, tag="attseg")
            att_seg[g] = seg
            pa_q, asc_q = [], []
            for bl in range(GB):
                bi = g * GB + bl
                Ssb, gel, v_sb = st.pop(bi)
                Sflat = Ssb.rearrange("p b r -> p (b r)")
                # sf-net output layer into a fresh PSUM, then S2 = S + h2 (DVE)
                h2_ps = ps_b.tile([L, 1024], F32, tag="psb")
                nc.tensor.matmul(h2_ps[:, 0:512], w2_sb, gel[:, 0:512],
                                 start=True, stop=True)
                nc.tensor.matmul(h2_ps[:, 512:GT], w2_sb, gel[:, 512:GT],
                                 start=True, stop=True)
                S2 = p_s2.tile([L, GT], BF16, tag="S2")
                nc.vector.tensor_add(S2, h2_ps[:, 0:GT], Sflat)
                # softmax numerator (no max subtraction; |scores2| < ~4)
                E_sb = p_esb.tile([L, GT], BF16, tag="E")
                nc.scalar.activation(out=E_sb, in_=S2, func=AF.Exp,
                                     bias=b2_sb, scale=1.0)
                st[bi] = (E_sb, v_sb)
                if bl >= 3:
                    asc_q.append(emit_norm(g, *pa_q.pop(0)))
                if bl >= 4:
                    emit_transpose(g, *asc_q.pop(0), seg)
                if bl >= 2:
                    pa_q.append(emit_attn(g, bl - 2, seg))
                # out-chunks on EVEN batches: their osb-copies then sit ahead
                # of the even exps in the scalar FIFO instead of delaying the
                # odd exps that gate the tail attns
                if out_group is not None and bl % 2 == 0 and bl >= 2:
                    emit_out_chunk(out_group, bl // 2 - 1)
            # tail: drain the stagger queues.  norm(b) must be emitted before
            # the attn that recycles pa(b)'s ring slot; real matmuls sit
            # between the transpose clusters so the HAM activity monitor
            # never sees an idle window, and the last out chunk carries the
            # PE across the B->A seam while the DVE tail frees the next
            # group's PSUM slots.
            pa_q.append(emit_attn(g, GB - 2, seg))          # attn(6)
            asc_q.append(emit_norm(g, *pa_q.pop(0)))        # norm(5)
            emit_transpose(g, *asc_q.pop(0), seg)           # tp(4)
            asc_q.append(emit_norm(g, *pa_q.pop(0)))        # norm(6)
            pa_q.append(emit_attn(g, GB - 1, seg))          # attn(7)
            asc_q.append(emit_norm(g, *pa_q.pop(0)))        # norm(7)
            emit_transpose(g, *asc_q.pop(0), seg)           # tp(5)
            emit_transpose(g, *asc_q.pop(0), seg)           # tp(6)
            if out_group is not None:
                emit_out_chunk(out_group, 3)
            emit_transpose(g, *asc_q.pop(0), seg)           # tp(7)

        def emit_norm(g, bl, pa):
            recip = p_small.tile([L, 2, 4], F32, tag="recip")
            # denominators live at col 512*bnk + 65*h + 64
            nc.vector.reciprocal(
                out=recip,
                in_=bass.AP(tensor=pa.tensor, offset=pa.offset + DH,
                            ap=[pa.ap[0], [512, 2], [DH + 1, 4]]))
            # scale + cast; bank b's blocks (heads 2*hh+b) go to col 128*hh+64*b
            # (single 4D-AP op: [part, bank, head, elem])
            asc = p_asc.tile([L, D], BF16, tag="asc")
            nc.vector.tensor_mul(
                bass.AP(tensor=asc.tensor, offset=asc.offset,
                        ap=[asc.ap[0], [DH, 2], [2 * DH, 4], [1, DH]]),
                bass.AP(tensor=pa.tensor, offset=pa.offset,
                        ap=[pa.ap[0], [512, 2], [DH + 1, 4], [1, DH]]),
                bass.AP(tensor=recip.tensor, offset=recip.offset,
                        ap=[recip.ap[0], [4, 2], [1, 4], [0, DH]]))
            return (bl, asc)

        def emit_transpose(g, bl, asc, seg):
            # transpose attn rows to [feat, tok] into the group attnT segment
            tp = ps_b.tile([128, 4, L + 1], BF16, tag="psb")
            for j in range(4):
                nc.tensor.transpose(tp[:, j, 0:L],
                                    asc[:, 128 * j:128 * (j + 1)], ident)
            nc.vector.tensor_copy(
                out=seg[:, :, bl * L:(bl + 1) * L], in_=tp[:, :, 0:L])

        def emit_out_chunk(g, dc):
            seg = att_seg[g]
            t0g = g * GT
            po = ps_a.tile([128, 1024], F32, tag="psa")
            for j in range(4):
                nc.tensor.matmul(
                    po[:, 0:512], Wo_sb[:, j, dc * 128:(dc + 1) * 128],
                    seg[:, j, 0:512], start=(j == 0), stop=(j == 3))
                nc.tensor.matmul(
                    po[:, 512:GT], Wo_sb[:, j, dc * 128:(dc + 1) * 128],
                    seg[:, j, 512:GT], start=(j == 0), stop=(j == 3))
            osb = p_osb.tile([128, GT], F32, tag="osb")
            nc.scalar.activation(out=osb, in_=po[:, 0:GT], func=AF.Identity,
                                 bias=boT_sb[:, dc:dc + 1], scale=1.0)
            nc.sync.dma_start(
                out=outT_d[dc * 128:(dc + 1) * 128, t0g:t0g + GT], in_=osb)
            if dc == 3:
                att_seg.pop(g)

        # ---- emission schedule ----
        # Group 0's phase A only needs q/k for tokens 0..568 (slabs 0-1) and
        # xtv(0): emit it right after slab 1 so the PE has ~14us of ready
        # work while slabs 2-8 stream in (the startup was DMA-feed-paced).
        emit_slab(0)
        emit_slab(1)
        ensure_xtv(0)
        ensure_xtv(1)
        w1_sb = singles.tile([L, FF], BF16, tag="w1")
        nc.gpsimd.dma_start(out=w1_sb, in_=w1)
        b1_sb = singles.tile([FF, 1], F32, tag="b1")
        nc.gpsimd.dma_start(out=b1_sb, in_=b1.rearrange("(p o) -> p o", o=1))
        emit_phase_a(0)
        for s in range(2, NSLAB):
            emit_slab(s)
        # constants not needed until phase B / out-proj
        Wo_sb = w_tiles(Wo, "o")
        w2_sb = singles.tile([FF, L], BF16, tag="w2")
        nc.gpsimd.dma_start(out=w2_sb, in_=w2)
        b2_sb = singles.tile([L, 1], F32, tag="b2")
        nc.gpsimd.dma_start(out=b2_sb, in_=b2.rearrange("(p o) -> p o", o=1))
        ident = singles.tile([L, L], BF16, tag="ident")
        make_identity(nc, ident)

        for g in range(G):
            if g + 1 < G:
                emit_phase_a(g + 1)
            emit_phase_b(g, out_group=(g - 1 if g >= 1 else None))
        for dc in range(4):
            emit_out_chunk(G - 1, dc)

    nc.compile()
    return nc


def _get_nc():
    if "nc" not in _CACHE:
        _CACHE["nc"] = _build()
    return _CACHE["nc"]


def _prep_in_maps(inputs):
    BF = ml_dtypes.bfloat16
    f32 = lambda a: np.asarray(a, dtype=np.float32)
    Wq_s = f32(inputs["Wq"]) * 0.125
    bq_s = f32(inputs["bq"]) * 0.125
    bo_eff = f32(inputs["bo"]) + f32(inputs["bv"]) @ f32(inputs["Wo"])
    shared = {
        "Wq": np.ascontiguousarray(Wq_s.astype(BF)),
        "Wk": np.ascontiguousarray(f32(inputs["Wk"]).astype(BF)),
        "Wv": np.ascontiguousarray(f32(inputs["Wv"]).astype(BF)),
        "Wo": np.ascontiguousarray(f32(inputs["Wo"]).astype(BF)),
        "bq": np.ascontiguousarray(bq_s),
        "bk": np.ascontiguousarray(f32(inputs["bk"])),
        "bo": np.ascontiguousarray(bo_eff),
        "sf_w1": np.ascontiguousarray(f32(inputs["sf_w1"]).astype(BF)),
        "sf_b1": np.ascontiguousarray(f32(inputs["sf_b1"])),
        "sf_w2": np.ascontiguousarray(f32(inputs["sf_w2"]).astype(BF)),
        "sf_b2": np.ascontiguousarray(f32(inputs["sf_b2"])),
    }
    xT = {}
    for key, name in (("query", "xqT"), ("key", "xkT"), ("value", "xvT")):
        # [B, L, D] -> [D, B, L] feature-major, bf16
        xT[name] = f32(inputs[key]).transpose(2, 0, 1).astype(BF)
    in_maps = []
    for c in range(N_CORES):
        m = dict(shared)
        for name in ("xqT", "xkT", "xvT"):
            m[name] = np.ascontiguousarray(
                xT[name][:, c * BC:(c + 1) * BC, :]).reshape(D, T)
        in_maps.append(m)
    return in_maps


def run(inputs, trace=False):
    nc = _get_nc()
    in_maps = _prep_in_maps(inputs)
    res = bass_utils.run_bass_kernel_spmd(
        nc, in_maps, core_ids=list(range(N_CORES)), trace=trace)
    out = np.concatenate(
        [np.asarray(res.results[c]["outT"], dtype=np.float32)
         .reshape(D, BC, L).transpose(1, 2, 0) for c in range(N_CORES)],
        axis=0)
    return out, res


def kernel(**inputs) -> np.ndarray:
    out, _ = run(inputs, trace=False)
    return out
